# revision 9
# baseline (speedup 1.0000x reference)
"""Multi-head attention on 8 Trainium2 NeuronCores (Bass/Tile, SPMD).

Problem: B=2, S=2048, d_model=128, n_heads=8, per-head dim 128.
    q/k/v = x @ W{q,k,v} + b   -> [B,S,H,128] -> heads
    attn  = softmax(q k^T / sqrt(128))  (mask is per-query-row)
    out   = concat_h(attn @ v) @ Wo + bo

Sharding: 16 (batch, head) pairs over 8 cores -> 2 heads of one batch per
core (data + head parallel).  Host sums the per-head partial outputs and
adds bias terms.

Per-core algorithm (S=2048, d=128, NJ=16 key tiles of 128):
  Q^T_h = Wq_h^T Xq^T, K^T_h likewise  (bf16 matmuls, f32 psum, bf16 evict)
  per head h, per key tile j:
    S^T_j = K_j^T Q with kj stationary serving all 2048 q (4 N=512 matmuls
      -> stationary loads amortized; weight loads are serialized with
      matmuls on this toolchain, so stationary reuse is the main lever)
    E_j = exp(S_j/sqrt(d) - 3) on ACT -> bf16 tiles (the -3 cancels in U/r)
    U_h += G_j^T E_j (bf16), deferred one tile behind scores/exp so the PE
      has filler while ACT catches up.  G_h = Xv @ Wv_h @ Wo_h is folded on
      the host, so U is the per-head output-projected context (unnorm.)
    DVE accumulates esum += E_j (bf16 2x mode) -> rowsums come from 4 tiny
      ones-matmuls over esum at the end (no per-tile PE rowsum pass)
  host: out[b] = sum_h U_h^T / r_h + (bv @ Wo + bo); masked rows fixed up.

All matmuls bf16 (1 cycle/col at N=512).  fp8 DoubleRow was tried and
rejected: per-weight fp8 quantization noise (~3%) transfers ~1:1 to the
softmax-weighted average (the signal is itself a ~1/sqrt(n_eff) average,
so the noise does not average down), giving ~4e-2 rel err vs the 2e-2
budget.  bf16 weights give ~5e-3.
"""

import os

import numpy as np

B = 2
S = 2048
D = 128
H = 8
P = 128
NCORES = 8
HPC = H * B // NCORES  # heads per core = 2
QC = 512               # query positions per matmul (psum bank limit)
NQC = S // QC          # 4
NJ = S // P            # 16 key-position tiles
SCALE = float(1.0 / np.sqrt(np.float32(D)))
EXPC = 3.0             # exp(s - EXPC); cancels in U/r
LN2 = float(np.log(2.0))
# DVE "fast exp": u16 = sat_round(s_raw*A + B) whose bits read back as bf16
# approximate 2^((s-EXPC)/ln2).  Used for N_FE of the 32 exp half-tiles per
# head to offload the ACT engine; ~4.5% per-weight ripple -> ~1.1% output
# err at N_FE=2 (consistent between U and r, so only the ripple matters).
FE_A = 128.0 * SCALE / LN2
FE_B = 16256.0 - 128.0 * EXPC / LN2
N_FE = 2

_CACHE = {}


def _build(with_sbias: bool, repeat: int = 1):
    import concourse.bacc as bacc
    import concourse.mybir as mybir
    from concourse.tile import TileContext

    F32 = mybir.dt.float32
    BF16 = mybir.dt.bfloat16
    U16 = mybir.dt.uint16
    EXP = mybir.ActivationFunctionType.Exp
    ADD = mybir.AluOpType.add
    MULT = mybir.AluOpType.mult

    nc = bacc.Bacc()
    xq = nc.declare_dram_parameter("xq_t", [P, S], BF16, isOutput=False)
    xk = nc.declare_dram_parameter("xk_t", [P, S], BF16, isOutput=False)
    wq = nc.declare_dram_parameter("wq", [P, HPC * P], BF16, isOutput=False)
    wk = nc.declare_dram_parameter("wk", [P, HPC * P], BF16, isOutput=False)
    # per-head G = Xv @ Wv_h @ Wo_h: g16[p, (h*NJ+j)*P + m] = G_h[j*P+p, m]
    g16 = nc.declare_dram_parameter("g16", [P, HPC * NJ * P], BF16,
                                    isOutput=False)
    sba_t_d = None
    if with_sbias:
        # per-(h, key tile) per-partition exp bias (bq != 0 path)
        sba_t_d = nc.declare_dram_parameter("sb_act", [P, HPC * NJ], F32,
                                            isOutput=False)
    out = nc.declare_dram_parameter("out_t", [HPC * P, S], F32, isOutput=True)
    rsum = nc.declare_dram_parameter("rsum", [HPC, S], F32, isOutput=True)

    with TileContext(nc) as tc:
        with (
            tc.tile_pool(name="const", bufs=1) as const,
            tc.tile_pool(name="qkv", bufs=1) as qkv,
            tc.tile_pool(name="exps", bufs=4) as exps,
            tc.tile_pool(name="esums", bufs=2) as esums,
            tc.tile_pool(name="small", bufs=3) as small,
            tc.tile_pool(name="ps_sc", bufs=2, space="PSUM") as ps_sc,
            tc.tile_pool(name="ps_acc", bufs=1, space="PSUM") as ps_acc,
        ):
            # ---- constants ----
            ones16 = const.tile([P, 1], BF16, tag="ones16")
            ones32 = small.tile([P, 1], F32, tag="ones32")
            nc.vector.memset(ones32[:], 1.0)
            nc.vector.tensor_copy(ones16[:], ones32[:])
            biasc = const.tile([P, 1], F32, tag="biasc")
            nc.vector.memset(biasc[:], -EXPC)

            # ---- load inputs (dma triggers spread over engines) ----
            wq_r = const.tile([P, HPC * P], BF16, tag="wq")
            wk_r = const.tile([P, HPC * P], BF16, tag="wk")
            nc.sync.dma_start(wq_r[:], wq[:])
            nc.gpsimd.dma_start(wk_r[:], wk[:])
            g16_r = const.tile([P, HPC * NJ * P], BF16, tag="g16")
            nc.scalar.dma_start(g16_r[:], g16[:])
            xq_r = const.tile([P, S], BF16, tag="xq")
            xk_r = const.tile([P, S], BF16, tag="xk")
            for c in range(0, NQC, 2):
                sl = slice(c * QC, (c + 2) * QC)
                nc.sync.dma_start(xq_r[:, sl], xq[:, sl])
                nc.gpsimd.dma_start(xk_r[:, sl], xk[:, sl])
            sba_t = None
            if with_sbias:
                sba_t = const.tile([P, HPC * NJ], F32, tag="sba")
                nc.sync.dma_start(sba_t[:], sba_t_d[:])

            import contextlib
            if repeat > 1:
                loop = tc.For_i(0, repeat, 1, hint_engines=(
                    mybir.EngineType.PE, mybir.EngineType.Activation,
                    mybir.EngineType.DVE, mybir.EngineType.SP))
            else:
                loop = contextlib.nullcontext()
            with loop:
                # ---- Q/K projections (bf16) ----
                qt, kt = [], []
                for h in range(HPC):
                    qt_h = qkv.tile([P, S], BF16, tag=f"qt{h}")
                    kt_h = qkv.tile([P, S], BF16, tag=f"kt{h}")
                    qt.append(qt_h)
                    kt.append(kt_h)
                    for w_r, x_r, t_h in ((wq_r, xq_r, qt_h),
                                          (wk_r, xk_r, kt_h)):
                        for c in range(0, NQC, 2):
                            sl = slice(c * QC, (c + 2) * QC)
                            pp = ps_sc.tile([P, 2 * QC], F32, tag="sc")
                            nc.tensor.matmul(pp[:, :QC],
                                             w_r[:, h * P:(h + 1) * P],
                                             x_r[:, c * QC:(c + 1) * QC],
                                             start=True, stop=True)
                            nc.tensor.matmul(pp[:, QC:],
                                             w_r[:, h * P:(h + 1) * P],
                                             x_r[:, (c + 1) * QC:(c + 2) * QC],
                                             start=True, stop=True)
                            nc.vector.tensor_copy(t_h[:, sl], pp[:])

                # ---- attention ----
                for h in range(HPC):
                    # U accumulator: whole head's q range (4 psum banks)
                    acc = ps_acc.tile([P, S], F32, tag="acc")

                    def av(j, et):
                        # U += G_j^T E_j, deferred one tile behind scores
                        gj = g16_r[:, (h * NJ + j) * P:(h * NJ + j + 1) * P]
                        st, sp = j == 0, j == NJ - 1
                        for c in range(NQC):
                            cs = slice(c * QC, (c + 1) * QC)
                            nc.tensor.matmul(acc[:, cs], gj, et[:, cs],
                                             start=st, stop=sp)

                    pend = []
                    esum = None
                    for j in range(NJ):
                        kj = kt[h][:, j * P:(j + 1) * P]
                        # scores for all 2048 q under one kj stationary
                        sc0 = ps_sc.tile([P, 2 * QC], F32, tag="sc")
                        sc1 = ps_sc.tile([P, 2 * QC], F32, tag="sc")
                        for c in range(NQC):
                            dst = sc0 if c < 2 else sc1
                            ds = slice((c % 2) * QC, (c % 2 + 1) * QC)
                            nc.tensor.matmul(dst[:, ds], kj,
                                             qt[h][:, c * QC:(c + 1) * QC],
                                             start=True, stop=True)
                        # exp -> bf16 tiles; ACT does most, DVE fast-exp
                        # takes the second half of every 4th tile
                        et = exps.tile([P, S], BF16, tag="et")
                        b = (sba_t[:, h * NJ + j:h * NJ + j + 1]
                             if with_sbias else biasc[:])
                        nc.scalar.activation(et[:, :2 * QC], sc0[:], EXP,
                                             bias=b, scale=SCALE)
                        if not with_sbias and N_FE and j % 4 == 3:
                            nc.vector.tensor_scalar(
                                et[:, 2 * QC:].bitcast(U16), sc1[:],
                                FE_A, FE_B, MULT, ADD)
                        else:
                            nc.scalar.activation(et[:, 2 * QC:], sc1[:], EXP,
                                                 bias=b, scale=SCALE)
                        # DVE: esum += E_j (bf16 2x mode), ping-pong
                        if j == 0:
                            e_prev = et
                        else:
                            e_new = esums.tile([P, S], BF16, tag="esum")
                            nc.vector.tensor_tensor(e_new[:], e_prev[:],
                                                    et[:], ADD)
                            e_prev = e_new
                        pend.append((j, et))
                        if len(pend) > 1:
                            av(*pend.pop(0))
                    for pj in pend:
                        av(*pj)
                    esum = e_prev

                    # rowsums: 4 ones-matmuls over esum -> psum row 0
                    rsa = ps_sc.tile([P, 2 * QC], F32, tag="sc")
                    rsb = ps_sc.tile([P, 2 * QC], F32, tag="sc")
                    for c in range(NQC):
                        rsp = rsa if c < 2 else rsb
                        ds = slice((c % 2) * QC, (c % 2 + 1) * QC)
                        nc.tensor.matmul(rsp[0:1, ds], ones16[:],
                                         esum[:, c * QC:(c + 1) * QC],
                                         start=True, stop=True)
                    rs_sb = small.tile([1, S], F32, tag="rs_sb")
                    nc.vector.tensor_copy(rs_sb[:, :2 * QC], rsa[0:1, :])
                    nc.vector.tensor_copy(rs_sb[:, 2 * QC:], rsb[0:1, :])
                    nc.sync.dma_start(rsum[h:h + 1, :], rs_sb[:])
                    # evict U^T and ship home
                    ot = small.tile([P, S], F32, tag=f"ot{h}")
                    nc.vector.tensor_copy(ot[:], acc[:])
                    nc.sync.dma_start(out[h * P:(h + 1) * P, :], ot[:])

    nc.compile()
    return nc


def _get_nc(with_sbias: bool):
    key = ("nc", with_sbias)
    if key not in _CACHE:
        _CACHE[key] = _build(with_sbias)
    return _CACHE[key]


def _to_bf16(a):
    import ml_dtypes
    return np.asarray(a, np.float32).astype(ml_dtypes.bfloat16)


def kernel(query, key, value, mask, Wq, bq, Wk, bk, Wv, bv, Wo, bo):
    from concourse.bass_utils import run_bass_kernel_spmd

    query = np.asarray(query, np.float32)
    key_ = np.asarray(key, np.float32)
    value = np.asarray(value, np.float32)
    mask = np.asarray(mask, bool)
    Wq, Wk, Wv, Wo = (np.asarray(a, np.float32) for a in (Wq, Wk, Wv, Wo))
    bq, bk, bv, bo = (np.asarray(a, np.float32) for a in (bq, bk, bv, bo))

    with_sbias = bool(np.any(bq != 0))
    nc = _get_nc(with_sbias)

    in_maps = []
    for c in range(NCORES):
        b = c // (NCORES // B)
        h0 = (c % (NCORES // B)) * HPC
        hs = slice(h0 * P, (h0 + HPC) * P)
        # G_h = Xv @ Wv_h @ Wo_h per head (output projection folded);
        # stationary layout [k-part within tile, (h, j, m)]
        g16 = np.concatenate(
            [np.ascontiguousarray(
                (value[b] @ Wv[:, (h0 + h) * P:(h0 + h + 1) * P]
                 @ Wo[(h0 + h) * P:(h0 + h + 1) * P, :])
                .reshape(NJ, P, P).transpose(1, 0, 2).reshape(P, NJ * P))
             for h in range(HPC)], axis=1)
        m = {
            "xq_t": _to_bf16(query[b].T),
            "xk_t": _to_bf16(key_[b].T),
            "wq": _to_bf16(Wq[:, hs]),
            "wk": _to_bf16(Wk[:, hs]),
            "g16": _to_bf16(g16),
        }
        if with_sbias:
            sb = np.zeros((P, HPC * NJ), np.float32)
            for h in range(HPC):
                col = Wk[:, (h0 + h) * P:(h0 + h + 1) * P] @ bq[
                    (h0 + h) * P:(h0 + h + 1) * P]
                v = (key_[b] @ col) * SCALE  # [S]
                sb[:, h * NJ:(h + 1) * NJ] = v.reshape(NJ, P).T
            m["sb_act"] = sb - EXPC
        in_maps.append(m)

    trace = os.environ.get("ATTN_TRACE") == "1"
    res = run_bass_kernel_spmd(nc, in_maps, list(range(NCORES)), trace=trace)
    _CACHE["last_result"] = res

    out = np.zeros((B, S, P), np.float32)
    for c in range(NCORES):
        b = c // (NCORES // B)
        ot = np.asarray(res.results[c]["out_t"])   # [HPC*P, S]
        rs = np.asarray(res.results[c]["rsum"])    # [HPC, S]
        for h in range(HPC):
            out[b] += ot[h * P:(h + 1) * P].T / rs[h][:, None]
    out += (bo + bv @ Wo)[None, None, :]

    if not mask.all():
        # masked query rows see a uniform distribution over all keys
        for b in range(B):
            bad = ~mask[b]
            if bad.any():
                ctx_u = value[b].mean(axis=0) @ Wv + bv  # [H*P]
                out[b, bad, :] = ctx_u @ Wo + bo
    return out.astype(np.float32)
